# revision 8
# baseline (speedup 1.0000x reference)
"""MLA attention (DeepSeek-style) distributed over 8 TRN2 NeuronCores.

Sharding: core c -> batch b=c//4, head-group/seq-shard g=c%4.
Phase A: down-proj of own 512-pos shard -> bf16 bounce -> 8-core AllGather.
Phase B: up-proj (Qt/Kt d-major, V row-major), flash-style causal attention
with St[kv,q] layout (softmax denominators via ones-matmul, no transposes),
AllGather of attention outputs, row-parallel wo on own s-shard.
"""

import numpy as np
import ml_dtypes

import concourse.bass as bass
import concourse.bacc as bacc
import concourse.tile as tile
import concourse.mybir as mybir
from concourse.bass_utils import run_bass_kernel_spmd

BF16 = ml_dtypes.bfloat16

# problem constants (hardcoded per harness rules)
DIM = 2048
N_HEADS = 16
Q_LORA = 1536
KV_LORA = 512
NOPE = 128
ROPE = 64
V_DIM = 128
QK_HD = NOPE + ROPE  # 192
EPS = 1e-6
B, S = 2, 2048
SCALE = QK_HD ** -0.5

NCORES = 8
GROUP = 4            # cores per batch
SSH = S // GROUP     # 512, seq shard
HPC = N_HEADS // GROUP  # 4 heads per core
P = 128
NKT = DIM // P       # 16 k-tiles of the model dim
NQM = Q_LORA // P    # 12
NKVM = KV_LORA // P  # 4
NCH = S // 512       # 4 s-chunks
DROWS = Q_LORA + KV_LORA + ROPE + 2  # 2114 bounce rows
GROWS = DROWS * NCORES

_cache = {}


def _build():
    nc = bacc.Bacc("TRN2", target_bir_lowering=False, debug=False,
                   num_devices=NCORES)
    f32 = mybir.dt.float32
    bf = mybir.dt.bfloat16
    i32 = mybir.dt.int32

    # ---- dram parameters (per-core values supplied via in_maps) ----
    xT = nc.dram_tensor("xT", [DIM, SSH], bf, kind="ExternalInput")
    wqaT = nc.dram_tensor("wqaT", [DIM, Q_LORA], bf, kind="ExternalInput")
    wkvaT = nc.dram_tensor("wkvaT", [DIM, KV_LORA + ROPE], bf,
                           kind="ExternalInput")
    wqbT = nc.dram_tensor("wqbT", [Q_LORA, HPC * QK_HD], bf,
                          kind="ExternalInput")
    wkvbT = nc.dram_tensor("wkvbT", [KV_LORA, HPC * (NOPE + V_DIM)], bf,
                           kind="ExternalInput")
    woT = nc.dram_tensor("woT", [N_HEADS * V_DIM, DIM], bf,
                         kind="ExternalInput")
    cos_sh = nc.dram_tensor("cos_sh", [ROPE, SSH], bf, kind="ExternalInput")
    sin_sh = nc.dram_tensor("sin_sh", [ROPE, SSH], bf, kind="ExternalInput")
    cos_full = nc.dram_tensor("cos_full", [ROPE, S], bf, kind="ExternalInput")
    sin_full = nc.dram_tensor("sin_full", [ROPE, S], bf, kind="ExternalInput")
    perm64 = nc.dram_tensor("perm64", [ROPE, ROPE], bf, kind="ExternalInput")
    trimask = nc.dram_tensor("trimask", [P, P], f32, kind="ExternalInput")
    cfg = nc.dram_tensor("cfg", [1, 4], i32, kind="ExternalInput")
    outT = nc.dram_tensor("out", [DIM, SSH], f32, kind="ExternalOutput")

    # ---- internal dram ----
    d_bounce = nc.dram_tensor("d_bounce", [DROWS, SSH], bf)
    d_gath = nc.dram_tensor("d_gath", [GROWS, SSH], bf, addr_space="Shared")
    o_bounce = nc.dram_tensor("o_bounce", [HPC * V_DIM, S], bf)
    o_gath = nc.dram_tensor("o_gath", [NCORES * HPC * V_DIM, S], bf,
                            addr_space="Shared")
    rg = [list(range(NCORES))]

    with tile.TileContext(nc) as tc:
        with (
            tc.tile_pool(name="persist", bufs=1) as persist,
            tc.tile_pool(name="attn", bufs=1) as attn_pool,
        ):
            # small persistent tiles
            ones_f = persist.tile([P, 1], f32)
            nc.vector.memset(ones_f, 1.0)
            ones_b = persist.tile([P, 1], bf)
            nc.vector.memset(ones_b, 1.0)
            mask_sb = persist.tile([P, P], f32)
            nc.sync.dma_start(out=mask_sb, in_=trimask[:])
            perm_sb = persist.tile([ROPE, ROPE], bf)
            nc.sync.dma_start(out=perm_sb, in_=perm64[:])
            cos_sh_sb = persist.tile([ROPE, SSH], bf)
            nc.sync.dma_start(out=cos_sh_sb, in_=cos_sh[:])
            sin_sh_sb = persist.tile([ROPE, SSH], bf)
            nc.sync.dma_start(out=sin_sh_sb, in_=sin_sh[:])
            cos_f_sb = persist.tile([ROPE, S], bf)
            nc.sync.dma_start(out=cos_f_sb, in_=cos_full[:])
            sin_f_sb = persist.tile([ROPE, S], bf)
            nc.sync.dma_start(out=sin_f_sb, in_=sin_full[:])
            eps_sb = persist.tile([1, 1], f32)
            nc.vector.memset(eps_sb, EPS)
            cfg_sb = persist.tile([1, 4], i32)
            nc.sync.dma_start(out=cfg_sb, in_=cfg[:])

            # per-core dynamic offsets
            r0 = nc.alloc_registers()
            nc.regs_load(r0, cfg_sb[0:1, 0:1])
            ag1_base = nc.snap(r0, donate=True, min_val=0,
                               max_val=GROUP * DROWS)
            r1 = nc.alloc_registers()
            nc.regs_load(r1, cfg_sb[0:1, 1:2])
            og_row = nc.snap(r1, donate=True, min_val=0, max_val=S)
            r2 = nc.alloc_registers()
            nc.regs_load(r2, cfg_sb[0:1, 2:3])
            og_col = nc.snap(r2, donate=True, min_val=0, max_val=S - 512)

            # attention-phase persistent tiles (filled by up-proj)
            qt_nope = [attn_pool.tile([P, S], bf, tag=f"qtn{h}",
                                      name=f"qt_nope{h}") for h in range(HPC)]
            qt_pe = [attn_pool.tile([ROPE, S], bf, tag=f"qtp{h}",
                                    name=f"qt_pe{h}") for h in range(HPC)]
            kt_nope = [attn_pool.tile([P, S], bf, tag=f"ktn{h}",
                                      name=f"kt_nope{h}") for h in range(HPC)]
            kpe_all = attn_pool.tile([ROPE, NCH, 512], bf)
            v_all = attn_pool.tile([P, S // P, HPC * V_DIM], bf)

            # ================= Phase A: down projections =================
            with (
                tc.tile_pool(name="pa", bufs=2) as pa,
                tc.tile_pool(name="pa_x", bufs=1) as pa_x,
                tc.tile_pool(name="pa_out", bufs=3) as pa_out,
                tc.tile_pool(name="pa_ps", bufs=2, space="PSUM") as pa_ps,
                tc.tile_pool(name="pa_st", bufs=1, space="PSUM") as pa_st,
            ):
                x_tiles = []
                for k in range(NKT):
                    t = pa_x.tile([P, SSH], bf, tag=f"x{k}")
                    nc.sync.dma_start(out=t, in_=xT[k * P:(k + 1) * P, :])
                    x_tiles.append(t)

                q_stat = pa_st.tile([1, SSH], f32)
                kv_stat = pa_st.tile([1, SSH], f32)

                def down_slab(wT, m0, mrows, dst_rows, stat_ps, stat_first,
                              stat_last, ev_tag="ev"):
                    """one output m-tile of a down projection + stats"""
                    slab = pa.tile([P, NKT, mrows], bf, tag="slab")
                    nc.sync.dma_start(
                        out=slab,
                        in_=wT[:, m0:m0 + mrows].rearrange(
                            "(kt p) m -> p kt m", p=P))
                    ps = pa_ps.tile([P, SSH], f32, tag="dps")
                    for k in range(NKT):
                        nc.tensor.matmul(ps[:mrows, :], slab[:, k, :],
                                         x_tiles[k], start=(k == 0),
                                         stop=(k == NKT - 1))
                    ev = pa_out.tile([P, SSH], bf, tag=ev_tag)
                    nc.vector.tensor_copy(ev[:mrows, :], ps[:mrows, :])
                    if dst_rows is not None:
                        nc.sync.dma_start(
                            out=d_bounce[dst_rows:dst_rows + mrows, :],
                            in_=ev[:mrows, :])
                    if stat_ps is not None:
                        sq = pa.tile([P, SSH], f32, tag="sq")
                        nc.scalar.square(sq[:mrows, :], ps[:mrows, :])
                        nc.tensor.matmul(stat_ps, ones_f[:mrows, :],
                                         sq[:mrows, :], start=stat_first,
                                         stop=stat_last)
                    return ps, ev

                for m in range(NQM):
                    down_slab(wqaT, m * P, P, m * P, q_stat, m == 0,
                              m == NQM - 1)
                for m in range(NKVM):
                    down_slab(wkvaT, m * P, P, Q_LORA + m * P, kv_stat,
                              m == 0, m == NKVM - 1)
                # k_pe slab + swapped k_pe slab (swap baked into host weights
                # would need 2x matmuls; use perm64 post-hoc instead)
                kpe_ps, kpe_ev = down_slab(wkvaT, KV_LORA, ROPE, None, None,
                                           False, False, ev_tag="kpe_ev")
                # rope: y = kpe * cos + (perm64.T @ kpe) * sin
                xs_ps = pa_ps.tile([ROPE, SSH], f32, tag="xs")
                nc.tensor.matmul(xs_ps, perm_sb, kpe_ev[:ROPE, :])
                y0 = pa.tile([ROPE, SSH], bf, tag="ry0")
                nc.vector.tensor_mul(y0, kpe_ev[:ROPE, :], cos_sh_sb)
                y1 = pa.tile([ROPE, SSH], bf, tag="ry1")
                nc.vector.tensor_mul(y1, xs_ps, sin_sh_sb)
                yr = pa.tile([ROPE, SSH], bf, tag="ryr")
                nc.vector.tensor_add(yr, y0, y1)
                nc.sync.dma_start(
                    out=d_bounce[Q_LORA + KV_LORA:Q_LORA + KV_LORA + ROPE, :],
                    in_=yr)

                # inv rms rows: 1/sqrt(mean(sq) + eps)
                for stat, n, row in ((q_stat, Q_LORA, DROWS - 2),
                                     (kv_stat, KV_LORA, DROWS - 1)):
                    tmp = pa.tile([1, SSH], f32, tag="srt")
                    nc.scalar.activation(tmp, stat,
                                         mybir.ActivationFunctionType.Sqrt,
                                         bias=eps_sb[0:1, 0:1], scale=1.0 / n)
                    rcp = pa.tile([1, SSH], f32, tag="rcp")
                    nc.vector.reciprocal(rcp, tmp)
                    rb = pa.tile([1, SSH], bf, tag="rb")
                    nc.vector.tensor_copy(rb, rcp)
                    nc.sync.dma_start(out=d_bounce[row:row + 1, :], in_=rb)

            # ================= AllGather 1 =================
            nc.gpsimd.collective_compute(
                "AllGather", mybir.AluOpType.bypass, replica_groups=rg,
                ins=[d_bounce[:]], outs=[d_gath[:]])

            # ================= Phase B: up projections =================
            with (
                tc.tile_pool(name="up_lat", bufs=1) as up_lat,
                tc.tile_pool(name="up_w", bufs=1) as up_w,
                tc.tile_pool(name="up", bufs=3) as up,
                tc.tile_pool(name="up_ps", bufs=3, space="PSUM") as up_ps,
                tc.tile_pool(name="pe_ps", bufs=2, space="PSUM") as pe_ps,
            ):
                # gathered latents for own batch (4 blocks -> 4 s-chunks)
                q_lat = up_lat.tile([P, NQM, NCH, 512], bf)
                kv_lat = up_lat.tile([P, NKVM, NCH, 512], bf)
                a_rows = up_lat.tile([1, 2, NCH, 512], bf)
                for r in range(NCH):
                    nc.sync.dma_start(
                        out=q_lat[:, :, r, :],
                        in_=d_gath[bass.ds(ag1_base + r * DROWS, Q_LORA), :]
                        .rearrange("(kt p) s -> p kt s", p=P))
                    nc.sync.dma_start(
                        out=kv_lat[:, :, r, :],
                        in_=d_gath[bass.ds(ag1_base + r * DROWS + Q_LORA,
                                           KV_LORA), :]
                        .rearrange("(kt p) s -> p kt s", p=P))
                    nc.sync.dma_start(
                        out=kpe_all[:, r, :],
                        in_=d_gath[bass.ds(
                            ag1_base + r * DROWS + Q_LORA + KV_LORA,
                            ROPE), :])
                    nc.sync.dma_start(
                        out=a_rows[0:1, :, r, :],
                        in_=d_gath[bass.ds(ag1_base + r * DROWS + DROWS - 2,
                                           2), :])

                # broadcast norm scales along partitions
                a_q_bc = up_lat.tile([P, NCH, 512], bf)
                a_kv_bc = up_lat.tile([P, NCH, 512], bf)
                for r in range(NCH):
                    nc.gpsimd.partition_broadcast(a_q_bc[:, r, :],
                                                  a_rows[0:1, 0, r, :])
                    nc.gpsimd.partition_broadcast(a_kv_bc[:, r, :],
                                                  a_rows[0:1, 1, r, :])
                # pre-scale kv latent by its inv-rms (q scaled at eviction)
                for k in range(NKVM):
                    for r in range(NCH):
                        nc.vector.tensor_mul(kv_lat[:, k, r, :],
                                             kv_lat[:, k, r, :],
                                             a_kv_bc[:, r, :])

                # weights
                wqb = up_w.tile([P, NQM, HPC * QK_HD], bf)
                nc.sync.dma_start(
                    out=wqb, in_=wqbT[:].rearrange("(kt p) m -> p kt m", p=P))
                wkvb = up_w.tile([P, NKVM, HPC * (NOPE + V_DIM)], bf)
                nc.sync.dma_start(
                    out=wkvb, in_=wkvbT[:].rearrange("(kt p) m -> p kt m",
                                                     p=P))

                # ---- q up: nope [4x128] then pe per head [64] ----
                for c in range(NCH):
                    for h in range(HPC):
                        ps = up_ps.tile([P, 512], f32, tag="up")
                        for k in range(NQM):
                            nc.tensor.matmul(
                                ps, wqb[:, k, h * P:(h + 1) * P],
                                q_lat[:, k, c, :], start=(k == 0),
                                stop=(k == NQM - 1))
                        nc.vector.tensor_mul(
                            qt_nope[h][:, c * 512:(c + 1) * 512], ps,
                            a_q_bc[:, c, :])
                    for h in range(HPC):
                        pcol = HPC * NOPE + h * ROPE
                        ps = pe_ps.tile([ROPE, 512], f32, tag="qp")
                        for k in range(NQM):
                            nc.tensor.matmul(
                                ps, wqb[:, k, pcol:pcol + ROPE],
                                q_lat[:, k, c, :], start=(k == 0),
                                stop=(k == NQM - 1))
                        pe_s = up.tile([ROPE, 512], bf, tag="pes")
                        nc.vector.tensor_mul(pe_s, ps, a_q_bc[:ROPE, c, :])
                        xs = pe_ps.tile([ROPE, 512], f32, tag="qpx")
                        nc.tensor.matmul(xs, perm_sb, pe_s)
                        dst = qt_pe[h][:, c * 512:(c + 1) * 512]
                        nc.vector.tensor_mul(dst, pe_s,
                                             cos_f_sb[:, c * 512:(c + 1) * 512])
                        t1 = up.tile([ROPE, 512], bf, tag="pet")
                        nc.vector.tensor_mul(t1, xs,
                                             sin_f_sb[:, c * 512:(c + 1) * 512])
                        nc.vector.tensor_add(dst, dst, t1)

                # ---- kv up: k_nope (d-major) and v (row-major) ----
                for c in range(NCH):
                    for h in range(HPC):
                        ps = up_ps.tile([P, 512], f32, tag="up")
                        for k in range(NKVM):
                            nc.tensor.matmul(
                                ps, wkvb[:, k, h * NOPE:(h + 1) * NOPE],
                                kv_lat[:, k, c, :], start=(k == 0),
                                stop=(k == NKVM - 1))
                        nc.vector.tensor_copy(
                            kt_nope[h][:, c * 512:(c + 1) * 512], ps)
                for sb in range(S // P):
                    c, part = sb // 4, sb % 4
                    ps = up_ps.tile([P, HPC * V_DIM], f32, tag="up")
                    for k in range(NKVM):
                        nc.tensor.matmul(
                            ps, kv_lat[:, k, c, part * P:(part + 1) * P],
                            wkvb[:, k, HPC * NOPE:], start=(k == 0),
                            stop=(k == NKVM - 1))
                    nc.vector.tensor_copy(v_all[:, sb, :], ps)

            # ================= attention =================
            with (
                tc.tile_pool(name="at", bufs=3) as at,
                tc.tile_pool(name="at_rl", bufs=2) as at_rl,
                tc.tile_pool(name="st_ps", bufs=2, space="PSUM") as st_ps,
                tc.tile_pool(name="ot_ps", bufs=2, space="PSUM") as ot_ps,
                tc.tile_pool(name="l_ps", bufs=2, space="PSUM") as l_ps,
            ):
                for h in range(HPC):
                    for qc in range(NCH):
                        nj = qc * 4 + 4
                        ot = ot_ps.tile([P, 512], f32, tag="ot")
                        lt = l_ps.tile([1, 512], f32, tag="l")
                        for j in range(nj):
                            d = j - qc * 4
                            off = max(0, d) * P
                            st = st_ps.tile([P, 512], f32, tag="st")
                            nc.tensor.matmul(
                                st[:, off:],
                                kt_nope[h][:, j * P:(j + 1) * P],
                                qt_nope[h][:, qc * 512 + off:(qc + 1) * 512],
                                start=True, stop=False)
                            nc.tensor.matmul(
                                st[:, off:],
                                kpe_all[:, j // 4, (j % 4) * P:(j % 4 + 1) * P],
                                qt_pe[h][:, qc * 512 + off:(qc + 1) * 512],
                                start=False, stop=True)
                            if d >= 0:
                                nc.vector.tensor_add(st[:, off:off + P],
                                                     st[:, off:off + P],
                                                     mask_sb)
                            pj = at.tile([P, 512], bf, tag="p")
                            nc.scalar.activation(
                                pj[:, off:], st[:, off:],
                                mybir.ActivationFunctionType.Exp)
                            nc.tensor.matmul(lt[:, off:], ones_b, pj[:, off:],
                                             start=(j == 0), stop=(j == nj - 1))
                            nc.tensor.matmul(
                                ot[:, off:],
                                v_all[:, j, h * V_DIM:(h + 1) * V_DIM],
                                pj[:, off:], start=(j == 0),
                                stop=(j == nj - 1))
                        rl = at_rl.tile([1, 512], f32, tag="rl")
                        nc.vector.reciprocal(rl, lt)
                        rlb = at_rl.tile([P, 512], f32, tag="rlb")
                        nc.gpsimd.partition_broadcast(rlb, rl)
                        ev = at.tile([P, 512], bf, tag="oev")
                        nc.vector.tensor_mul(ev, ot, rlb)
                        nc.sync.dma_start(
                            out=o_bounce[h * V_DIM:(h + 1) * V_DIM,
                                         qc * 512:(qc + 1) * 512],
                            in_=ev)

            # ================= AllGather 2 =================
            nc.gpsimd.collective_compute(
                "AllGather", mybir.AluOpType.bypass, replica_groups=rg,
                ins=[o_bounce[:]], outs=[o_gath[:]])

            # ================= wo (row-parallel, own s-shard) =================
            with (
                tc.tile_pool(name="wo_rhs", bufs=1) as wo_rhs,
                tc.tile_pool(name="wo_w", bufs=3) as wo_w,
                tc.tile_pool(name="wo_ev", bufs=3) as wo_ev,
                tc.tile_pool(name="wo_ps", bufs=2, space="PSUM") as wo_ps,
            ):
                rhs = wo_rhs.tile([P, NKT, 512], bf)
                for k in range(NKT):
                    nc.sync.dma_start(
                        out=rhs[:, k, :],
                        in_=o_gath[bass.ds(og_row + k * P, P),
                                   bass.ds(og_col, 512)])
                for m in range(NKT):
                    slab = wo_w.tile([P, NKT, P], bf, tag="woslab")
                    nc.sync.dma_start(
                        out=slab,
                        in_=woT[:, m * P:(m + 1) * P].rearrange(
                            "(kt p) m -> p kt m", p=P))
                    ps = wo_ps.tile([P, 512], f32, tag="wops")
                    for k in range(NKT):
                        nc.tensor.matmul(ps, slab[:, k, :], rhs[:, k, :],
                                         start=(k == 0), stop=(k == NKT - 1))
                    ev = wo_ev.tile([P, 512], f32, tag="woev")
                    nc.vector.tensor_copy(ev, ps)
                    nc.sync.dma_start(out=outT[m * P:(m + 1) * P, :], in_=ev)

    nc.compile()
    return nc


def _prep_inputs(x, freqs_cos, freqs_sin, wq_a, q_norm_w, wq_b, wkv_a,
                 kv_norm_w, wkv_b, wo):
    x = np.asarray(x, np.float32)
    freqs_cos = np.asarray(freqs_cos, np.float32)
    freqs_sin = np.asarray(freqs_sin, np.float32)
    wq_a = np.asarray(wq_a, np.float32)
    q_norm_w = np.asarray(q_norm_w, np.float32)
    wq_b = np.asarray(wq_b, np.float32)
    wkv_a = np.asarray(wkv_a, np.float32)
    kv_norm_w = np.asarray(kv_norm_w, np.float32)
    wkv_b = np.asarray(wkv_b, np.float32)
    wo = np.asarray(wo, np.float32)

    wqaT = np.ascontiguousarray(wq_a.T).astype(BF16)
    # wkv_a rows: [kv_lora latent | rope k_pe]; append swapped+signed pe rows
    # is NOT needed (perm64 approach) -> wkvaT = plain transpose
    wkvaT = np.ascontiguousarray(wkv_a.T).astype(BF16)

    # fold q_norm_w and softmax scale into wq_b; reorder columns per core:
    # [h0..h3 nope (4x128) | h0..h3 pe (4x64)]
    wqb_eff = (wq_b * q_norm_w[None, :]) * SCALE  # [H*192, Q_LORA]
    wqb_eff = wqb_eff.reshape(N_HEADS, QK_HD, Q_LORA)
    wkvb_eff = wkv_b * kv_norm_w[None, :]
    wkvb_eff = wkvb_eff.reshape(N_HEADS, NOPE + V_DIM, KV_LORA)

    # rope tables expanded to [64, S] (row 2i and 2i+1 share freq i)
    cosT = np.repeat(freqs_cos.T, 2, axis=0)  # [64, S]
    sinT = np.repeat(freqs_sin.T, 2, axis=0)

    perm = np.zeros((ROPE, ROPE), np.float32)
    for i in range(ROPE // 2):
        perm[2 * i + 1, 2 * i] = -1.0  # out[2i]   = -x[2i+1]
        perm[2 * i, 2 * i + 1] = 1.0   # out[2i+1] =  x[2i]
    r = np.arange(P)
    trimask = np.where(r[:, None] <= r[None, :], 0.0, -1e30).astype(np.float32)

    in_maps = []
    for c in range(NCORES):
        b, g = c // GROUP, c % GROUP
        heads = slice(g * HPC, (g + 1) * HPC)
        xT = np.ascontiguousarray(
            x[b].T[:, g * SSH:(g + 1) * SSH]).astype(BF16)
        wqbT = np.concatenate(
            [wqb_eff[heads, :NOPE].reshape(HPC * NOPE, Q_LORA),
             wqb_eff[heads, NOPE:].reshape(HPC * ROPE, Q_LORA)],
            axis=0).T  # [Q_LORA, 768]
        wkvbT = np.concatenate(
            [wkvb_eff[heads, :NOPE].reshape(HPC * NOPE, KV_LORA),
             wkvb_eff[heads, NOPE:].reshape(HPC * V_DIM, KV_LORA)],
            axis=0).T  # [KV_LORA, 1024]
        in_maps.append({
            "xT": xT,
            "wqaT": wqaT,
            "wkvaT": wkvaT,
            "wqbT": np.ascontiguousarray(wqbT).astype(BF16),
            "wkvbT": np.ascontiguousarray(wkvbT).astype(BF16),
            "woT": np.ascontiguousarray(wo.T).astype(BF16),
            "cos_sh": np.ascontiguousarray(
                cosT[:, g * SSH:(g + 1) * SSH]).astype(BF16),
            "sin_sh": np.ascontiguousarray(
                sinT[:, g * SSH:(g + 1) * SSH]).astype(BF16),
            "cos_full": np.ascontiguousarray(cosT).astype(BF16),
            "sin_full": np.ascontiguousarray(sinT).astype(BF16),
            "perm64": perm.astype(BF16),
            "trimask": trimask,
            "cfg": np.array([[b * GROUP * DROWS, b * S, g * 512, 0]],
                            np.int32),
        })
    return in_maps


def _run(inputs, trace=False, **kw):
    if "nc" not in _cache:
        _cache["nc"] = _build()
    nc = _cache["nc"]
    in_maps = _prep_inputs(**inputs)
    res = run_bass_kernel_spmd(nc, in_maps, core_ids=list(range(NCORES)),
                               trace=trace, **kw)
    out = np.empty((B, S, DIM), np.float32)
    for c in range(NCORES):
        b, g = c // GROUP, c % GROUP
        out[b, g * SSH:(g + 1) * SSH, :] = res.results[c]["out"].T
    return out, res


def kernel(**inputs):
    out, _ = _run(inputs)
    return out


# revision 9
# speedup vs baseline: 1.1439x; 1.1439x over previous
"""MLA attention (DeepSeek-style) distributed over 8 TRN2 NeuronCores.

Sharding: core c -> batch b=c//4, head-group/seq-shard g=c%4.
Phase A: down-proj of own 512-pos shard -> bf16 bounce -> 8-core AllGathers
(kv first, then q, so comms overlap the other projection's compute).
Phase B: up-proj (Qt/Kt d-major, V row-major), flash-style causal attention
with St[kv,q] layout (softmax denominators via ones-matmul, no transposes),
head-pair-split AllGathers of attention outputs overlapping attention and
the first half of the row-parallel wo matmul on own s-shard.
"""

import numpy as np
import ml_dtypes

import concourse.bass as bass
import concourse.bacc as bacc
import concourse.tile as tile
import concourse.mybir as mybir
from concourse.bass_utils import run_bass_kernel_spmd

BF16 = ml_dtypes.bfloat16

# problem constants (hardcoded per harness rules)
DIM = 2048
N_HEADS = 16
Q_LORA = 1536
KV_LORA = 512
NOPE = 128
ROPE = 64
V_DIM = 128
QK_HD = NOPE + ROPE  # 192
EPS = 1e-6
B, S = 2, 2048
SCALE = QK_HD ** -0.5

NCORES = 8
GROUP = 4               # cores per batch
SSH = S // GROUP        # 512, seq shard
HPC = N_HEADS // GROUP  # 4 heads per core
P = 128
NKT = DIM // P          # 16
NQM = Q_LORA // P       # 12
NKVM = KV_LORA // P     # 4
NCH = S // 512          # 4
KVROWS = KV_LORA + ROPE + 1   # 577 kv bounce rows
QROWS = Q_LORA + 1            # 1537 q bounce rows

_cache = {}


def _build():
    nc = bacc.Bacc("TRN2", target_bir_lowering=False, debug=False,
                   num_devices=NCORES)
    f32 = mybir.dt.float32
    bf = mybir.dt.bfloat16
    i32 = mybir.dt.int32

    # ---- dram parameters ----
    xT = nc.dram_tensor("xT", [DIM, SSH], bf, kind="ExternalInput")
    wqaT = nc.dram_tensor("wqaT", [DIM, Q_LORA], bf, kind="ExternalInput")
    wkvaT = nc.dram_tensor("wkvaT", [DIM, KV_LORA + ROPE], bf,
                           kind="ExternalInput")
    wqbT = nc.dram_tensor("wqbT", [Q_LORA, HPC * QK_HD], bf,
                          kind="ExternalInput")
    wkvbT = nc.dram_tensor("wkvbT", [KV_LORA, HPC * (NOPE + V_DIM)], bf,
                           kind="ExternalInput")
    woTa = nc.dram_tensor("woTa", [N_HEADS * V_DIM // 2, DIM], bf,
                          kind="ExternalInput")
    woTb = nc.dram_tensor("woTb", [N_HEADS * V_DIM // 2, DIM], bf,
                          kind="ExternalInput")
    cos_sh = nc.dram_tensor("cos_sh", [ROPE, SSH], bf, kind="ExternalInput")
    sin_sh = nc.dram_tensor("sin_sh", [ROPE, SSH], bf, kind="ExternalInput")
    cos_full = nc.dram_tensor("cos_full", [ROPE, S], bf, kind="ExternalInput")
    sin_full = nc.dram_tensor("sin_full", [ROPE, S], bf, kind="ExternalInput")
    perm64 = nc.dram_tensor("perm64", [ROPE, ROPE], bf, kind="ExternalInput")
    trimask = nc.dram_tensor("trimask", [P, P], f32, kind="ExternalInput")
    cfg = nc.dram_tensor("cfg", [1, 4], i32, kind="ExternalInput")
    outT = nc.dram_tensor("out", [DIM, SSH], f32, kind="ExternalOutput")

    # ---- internal dram ----
    b_kv = nc.dram_tensor("b_kv", [KVROWS, SSH], bf)
    g_kv = nc.dram_tensor("g_kv", [NCORES * KVROWS, SSH], bf,
                          addr_space="Shared")
    b_q = nc.dram_tensor("b_q", [QROWS, SSH], bf)
    g_q = nc.dram_tensor("g_q", [NCORES * QROWS, SSH], bf,
                         addr_space="Shared")
    o_bnc = [nc.dram_tensor(f"o_bnc{i}", [2 * V_DIM, S], bf)
             for i in range(2)]
    o_g = [nc.dram_tensor(f"o_g{i}", [NCORES * 2 * V_DIM, S], bf,
                          addr_space="Shared") for i in range(2)]
    rg = [list(range(NCORES))]

    with tile.TileContext(nc) as tc:
        with (
            tc.tile_pool(name="persist", bufs=1) as persist,
            tc.tile_pool(name="attn", bufs=1) as attn_pool,
        ):
            # small persistent tiles
            ones_f = persist.tile([P, 1], f32)
            nc.vector.memset(ones_f, 1.0)
            ones_b = persist.tile([P, 1], bf)
            nc.vector.memset(ones_b, 1.0)
            eps_sb = persist.tile([1, 1], f32)
            nc.vector.memset(eps_sb, EPS)
            mask_sb = persist.tile([P, P], f32)
            nc.sync.dma_start(out=mask_sb, in_=trimask[:])
            perm_sb = persist.tile([ROPE, ROPE], bf)
            nc.sync.dma_start(out=perm_sb, in_=perm64[:])
            cos_sh_sb = persist.tile([ROPE, SSH], bf)
            nc.sync.dma_start(out=cos_sh_sb, in_=cos_sh[:])
            sin_sh_sb = persist.tile([ROPE, SSH], bf)
            nc.sync.dma_start(out=sin_sh_sb, in_=sin_sh[:])
            cos_f_sb = persist.tile([ROPE, S], bf)
            nc.sync.dma_start(out=cos_f_sb, in_=cos_full[:])
            sin_f_sb = persist.tile([ROPE, S], bf)
            nc.sync.dma_start(out=sin_f_sb, in_=sin_full[:])
            cfg_sb = persist.tile([1, 4], i32)
            nc.sync.dma_start(out=cfg_sb, in_=cfg[:])

            # per-core dynamic offsets
            r0 = nc.alloc_registers()
            nc.regs_load(r0, cfg_sb[0:1, 0:1])
            kv_base = nc.snap(r0, donate=True, min_val=0,
                              max_val=GROUP * KVROWS)
            r1 = nc.alloc_registers()
            nc.regs_load(r1, cfg_sb[0:1, 1:2])
            q_base = nc.snap(r1, donate=True, min_val=0,
                             max_val=GROUP * QROWS)
            r2 = nc.alloc_registers()
            nc.regs_load(r2, cfg_sb[0:1, 2:3])
            og_row = nc.snap(r2, donate=True, min_val=0, max_val=S // 2)
            r3 = nc.alloc_registers()
            nc.regs_load(r3, cfg_sb[0:1, 3:4])
            og_col = nc.snap(r3, donate=True, min_val=0, max_val=S - 512)

            # attention-phase persistent tiles (filled by up-proj)
            qt_nope = [attn_pool.tile([P, S], bf, tag=f"qtn{h}",
                                      name=f"qt_nope{h}") for h in range(HPC)]
            qt_pe = [attn_pool.tile([ROPE, S], bf, tag=f"qtp{h}",
                                    name=f"qt_pe{h}") for h in range(HPC)]
            kt_nope = [attn_pool.tile([P, S], bf, tag=f"ktn{h}",
                                      name=f"kt_nope{h}") for h in range(HPC)]
            kpe_all = attn_pool.tile([ROPE, NCH, 512], bf)
            v_all = attn_pool.tile([P, S // P, HPC * V_DIM], bf)

            # ================= Phase A: down projections =================
            with (
                tc.tile_pool(name="pa", bufs=2) as pa,
                tc.tile_pool(name="pa_x", bufs=1) as pa_x,
                tc.tile_pool(name="pa_out", bufs=3) as pa_out,
                tc.tile_pool(name="pa_ps", bufs=2, space="PSUM") as pa_ps,
                tc.tile_pool(name="pa_st", bufs=1, space="PSUM") as pa_st,
            ):
                x_all = pa_x.tile([P, NKT, SSH], bf)
                nc.sync.dma_start(
                    out=x_all, in_=xT[:].rearrange("(kt p) s -> p kt s", p=P))

                q_stat = pa_st.tile([1, SSH], f32)
                kv_stat = pa_st.tile([1, SSH], f32)

                def down_slab(wT, m0, mrows, bounce, dst_rows, stat_ps,
                              stat_first, stat_last, ev_tag="ev"):
                    slab = pa.tile([P, NKT, mrows], bf, tag="slab")
                    nc.sync.dma_start(
                        out=slab,
                        in_=wT[:, m0:m0 + mrows].rearrange(
                            "(kt p) m -> p kt m", p=P))
                    ps = pa_ps.tile([P, SSH], f32, tag="dps")
                    for k in range(NKT):
                        nc.tensor.matmul(ps[:mrows, :], slab[:, k, :],
                                         x_all[:, k, :], start=(k == 0),
                                         stop=(k == NKT - 1))
                    ev = pa_out.tile([P, SSH], bf, tag=ev_tag)
                    nc.vector.tensor_copy(ev[:mrows, :], ps[:mrows, :])
                    if dst_rows is not None:
                        nc.sync.dma_start(
                            out=bounce[dst_rows:dst_rows + mrows, :],
                            in_=ev[:mrows, :])
                    if stat_ps is not None:
                        sq = pa.tile([P, SSH], f32, tag="sq")
                        nc.scalar.square(sq[:mrows, :], ps[:mrows, :])
                        nc.tensor.matmul(stat_ps, ones_f[:mrows, :],
                                         sq[:mrows, :], start=stat_first,
                                         stop=stat_last)
                    return ev

                def stat_row(stat, n, bounce, row):
                    tmp = pa.tile([1, SSH], f32, tag="srt")
                    nc.scalar.activation(tmp, stat,
                                         mybir.ActivationFunctionType.Sqrt,
                                         bias=eps_sb[0:1, 0:1], scale=1.0 / n)
                    rcp = pa.tile([1, SSH], f32, tag="rcp")
                    nc.vector.reciprocal(rcp, tmp)
                    rb = pa.tile([1, SSH], bf, tag="rb")
                    nc.vector.tensor_copy(rb, rcp)
                    nc.sync.dma_start(out=bounce[row:row + 1, :], in_=rb)

                # ---- kv first (so AG_kv overlaps q down-proj) ----
                for m in range(NKVM):
                    down_slab(wkvaT, m * P, P, b_kv, m * P, kv_stat,
                              m == 0, m == NKVM - 1)
                kpe_ev = down_slab(wkvaT, KV_LORA, ROPE, None, None, None,
                                   False, False, ev_tag="kpe_ev")
                xs_ps = pa_ps.tile([ROPE, SSH], f32, tag="xs")
                nc.tensor.matmul(xs_ps, perm_sb, kpe_ev[:ROPE, :])
                y0 = pa.tile([ROPE, SSH], bf, tag="ry0")
                nc.vector.tensor_mul(y0, kpe_ev[:ROPE, :], cos_sh_sb)
                y1 = pa.tile([ROPE, SSH], bf, tag="ry1")
                nc.vector.tensor_mul(y1, xs_ps, sin_sh_sb)
                yr = pa.tile([ROPE, SSH], bf, tag="ryr")
                nc.vector.tensor_add(yr, y0, y1)
                nc.sync.dma_start(out=b_kv[KV_LORA:KV_LORA + ROPE, :], in_=yr)
                stat_row(kv_stat, KV_LORA, b_kv, KVROWS - 1)

                nc.gpsimd.collective_compute(
                    "AllGather", mybir.AluOpType.bypass, replica_groups=rg,
                    ins=[b_kv[:]], outs=[g_kv[:]])

                # ---- q down-proj (overlaps AG_kv) ----
                for m in range(NQM):
                    down_slab(wqaT, m * P, P, b_q, m * P, q_stat,
                              m == 0, m == NQM - 1)
                stat_row(q_stat, Q_LORA, b_q, QROWS - 1)

                nc.gpsimd.collective_compute(
                    "AllGather", mybir.AluOpType.bypass, replica_groups=rg,
                    ins=[b_q[:]], outs=[g_q[:]])

            # ================= Phase B =================
            with (
                tc.tile_pool(name="up_lat", bufs=1) as up_lat,
                tc.tile_pool(name="up_w", bufs=1) as up_w,
                tc.tile_pool(name="up", bufs=3) as up,
                tc.tile_pool(name="up_ps", bufs=3, space="PSUM") as up_ps,
                tc.tile_pool(name="pe_ps", bufs=2, space="PSUM") as pe_ps,
            ):
                # ---- kv consumption (waits only on AG_kv) ----
                kv_lat = up_lat.tile([P, NKVM, NCH, 512], bf)
                akv_row = up_lat.tile([1, NCH, 512], bf)
                for r in range(NCH):
                    nc.sync.dma_start(
                        out=kv_lat[:, :, r, :],
                        in_=g_kv[bass.ds(kv_base + r * KVROWS, KV_LORA), :]
                        .rearrange("(kt p) s -> p kt s", p=P))
                    nc.sync.dma_start(
                        out=kpe_all[:, r, :],
                        in_=g_kv[bass.ds(kv_base + r * KVROWS + KV_LORA,
                                         ROPE), :])
                    nc.sync.dma_start(
                        out=akv_row[0:1, r, :],
                        in_=g_kv[bass.ds(kv_base + r * KVROWS + KVROWS - 1,
                                         1), :])
                a_kv_bc = up_lat.tile([P, NCH, 512], bf)
                for r in range(NCH):
                    nc.gpsimd.partition_broadcast(a_kv_bc[:, r, :],
                                                  akv_row[0:1, r, :])
                for k in range(NKVM):
                    for r in range(NCH):
                        nc.vector.tensor_mul(kv_lat[:, k, r, :],
                                             kv_lat[:, k, r, :],
                                             a_kv_bc[:, r, :])
                wkvb = up_w.tile([P, NKVM, HPC * (NOPE + V_DIM)], bf)
                nc.sync.dma_start(
                    out=wkvb, in_=wkvbT[:].rearrange("(kt p) m -> p kt m",
                                                     p=P))
                # k_nope (d-major) + v (row-major)
                for c in range(NCH):
                    for h in range(HPC):
                        ps = up_ps.tile([P, 512], f32, tag="up")
                        for k in range(NKVM):
                            nc.tensor.matmul(
                                ps, wkvb[:, k, h * NOPE:(h + 1) * NOPE],
                                kv_lat[:, k, c, :], start=(k == 0),
                                stop=(k == NKVM - 1))
                        nc.vector.tensor_copy(
                            kt_nope[h][:, c * 512:(c + 1) * 512], ps)
                for sb in range(S // P):
                    c, part = sb // 4, sb % 4
                    ps = up_ps.tile([P, HPC * V_DIM], f32, tag="up")
                    for k in range(NKVM):
                        nc.tensor.matmul(
                            ps, kv_lat[:, k, c, part * P:(part + 1) * P],
                            wkvb[:, k, HPC * NOPE:], start=(k == 0),
                            stop=(k == NKVM - 1))
                    nc.vector.tensor_copy(v_all[:, sb, :], ps)

                # ---- q consumption (waits on AG_q) ----
                q_lat = up_lat.tile([P, NQM, NCH, 512], bf)
                aq_row = up_lat.tile([1, NCH, 512], bf)
                for r in range(NCH):
                    nc.sync.dma_start(
                        out=q_lat[:, :, r, :],
                        in_=g_q[bass.ds(q_base + r * QROWS, Q_LORA), :]
                        .rearrange("(kt p) s -> p kt s", p=P))
                    nc.sync.dma_start(
                        out=aq_row[0:1, r, :],
                        in_=g_q[bass.ds(q_base + r * QROWS + QROWS - 1,
                                        1), :])
                a_q_bc = up_lat.tile([P, NCH, 512], bf)
                for r in range(NCH):
                    nc.gpsimd.partition_broadcast(a_q_bc[:, r, :],
                                                  aq_row[0:1, r, :])
                wqb = up_w.tile([P, NQM, HPC * QK_HD], bf)
                nc.sync.dma_start(
                    out=wqb, in_=wqbT[:].rearrange("(kt p) m -> p kt m", p=P))
                for c in range(NCH):
                    for h in range(HPC):
                        ps = up_ps.tile([P, 512], f32, tag="up")
                        for k in range(NQM):
                            nc.tensor.matmul(
                                ps, wqb[:, k, h * P:(h + 1) * P],
                                q_lat[:, k, c, :], start=(k == 0),
                                stop=(k == NQM - 1))
                        nc.vector.tensor_mul(
                            qt_nope[h][:, c * 512:(c + 1) * 512], ps,
                            a_q_bc[:, c, :])
                    for h in range(HPC):
                        pcol = HPC * NOPE + h * ROPE
                        ps = pe_ps.tile([ROPE, 512], f32, tag="qp")
                        for k in range(NQM):
                            nc.tensor.matmul(
                                ps, wqb[:, k, pcol:pcol + ROPE],
                                q_lat[:, k, c, :], start=(k == 0),
                                stop=(k == NQM - 1))
                        pe_s = up.tile([ROPE, 512], bf, tag="pes")
                        nc.vector.tensor_mul(pe_s, ps, a_q_bc[:ROPE, c, :])
                        xs = pe_ps.tile([ROPE, 512], f32, tag="qpx")
                        nc.tensor.matmul(xs, perm_sb, pe_s)
                        dst = qt_pe[h][:, c * 512:(c + 1) * 512]
                        nc.vector.tensor_mul(
                            dst, pe_s, cos_f_sb[:, c * 512:(c + 1) * 512])
                        t1 = up.tile([ROPE, 512], bf, tag="pet")
                        nc.vector.tensor_mul(
                            t1, xs, sin_f_sb[:, c * 512:(c + 1) * 512])
                        nc.vector.tensor_add(dst, dst, t1)

            # ================= attention (+ split output AGs) =============
            with (
                tc.tile_pool(name="at", bufs=3) as at,
                tc.tile_pool(name="at_rl", bufs=2) as at_rl,
                tc.tile_pool(name="st_ps", bufs=2, space="PSUM") as st_ps,
                tc.tile_pool(name="ot_ps", bufs=2, space="PSUM") as ot_ps,
                tc.tile_pool(name="l_ps", bufs=2, space="PSUM") as l_ps,
            ):
                for h in range(HPC):
                    for qc in range(NCH):
                        nj = qc * 4 + 4
                        ot = ot_ps.tile([P, 512], f32, tag="ot")
                        lt = l_ps.tile([1, 512], f32, tag="l")
                        for j in range(nj):
                            d = j - qc * 4
                            off = max(0, d) * P
                            st = st_ps.tile([P, 512], f32, tag="st")
                            nc.tensor.matmul(
                                st[:, off:],
                                kt_nope[h][:, j * P:(j + 1) * P],
                                qt_nope[h][:, qc * 512 + off:(qc + 1) * 512],
                                start=True, stop=False)
                            nc.tensor.matmul(
                                st[:, off:],
                                kpe_all[:, j // 4,
                                        (j % 4) * P:(j % 4 + 1) * P],
                                qt_pe[h][:, qc * 512 + off:(qc + 1) * 512],
                                start=False, stop=True)
                            if d >= 0:
                                nc.vector.tensor_add(st[:, off:off + P],
                                                     st[:, off:off + P],
                                                     mask_sb)
                            pj = at.tile([P, 512], bf, tag="p")
                            nc.scalar.activation(
                                pj[:, off:], st[:, off:],
                                mybir.ActivationFunctionType.Exp)
                            nc.tensor.matmul(lt[:, off:], ones_b,
                                             pj[:, off:], start=(j == 0),
                                             stop=(j == nj - 1))
                            nc.tensor.matmul(
                                ot[:, off:],
                                v_all[:, j, h * V_DIM:(h + 1) * V_DIM],
                                pj[:, off:], start=(j == 0),
                                stop=(j == nj - 1))
                        rl = at_rl.tile([1, 512], f32, tag="rl")
                        nc.vector.reciprocal(rl, lt)
                        rlb = at_rl.tile([P, 512], f32, tag="rlb")
                        nc.gpsimd.partition_broadcast(rlb, rl)
                        ev = at.tile([P, 512], bf, tag="oev")
                        nc.vector.tensor_mul(ev, ot, rlb)
                        nc.sync.dma_start(
                            out=o_bnc[h // 2][(h % 2) * V_DIM:
                                              (h % 2 + 1) * V_DIM,
                                              qc * 512:(qc + 1) * 512],
                            in_=ev)
                    if h % 2 == 1:
                        nc.gpsimd.collective_compute(
                            "AllGather", mybir.AluOpType.bypass,
                            replica_groups=rg, ins=[o_bnc[h // 2][:]],
                            outs=[o_g[h // 2][:]])

            # ================= wo (two passes over head-pair gathers) ======
            with (
                tc.tile_pool(name="wo_rhs", bufs=1) as wo_rhs,
                tc.tile_pool(name="wo_acc", bufs=1) as wo_acc,
                tc.tile_pool(name="wo_w", bufs=3) as wo_w,
                tc.tile_pool(name="wo_ev", bufs=3) as wo_ev,
                tc.tile_pool(name="wo_ps", bufs=2, space="PSUM") as wo_ps,
            ):
                NH2 = NKT // 2  # 8 k-tiles per pass
                acc = wo_acc.tile([P, NKT, 512], f32)
                for half, woX in ((0, woTa), (1, woTb)):
                    rhs = wo_rhs.tile([P, NH2, 512], bf, tag=f"rhs{half}",
                                      name=f"rhs{half}")
                    for k in range(NH2):
                        nc.sync.dma_start(
                            out=rhs[:, k, :],
                            in_=o_g[half][bass.ds(og_row + k * P, P),
                                          bass.ds(og_col, 512)])
                    for m in range(NKT):
                        slab = wo_w.tile([P, NH2, P], bf, tag="woslab")
                        nc.sync.dma_start(
                            out=slab,
                            in_=woX[:, m * P:(m + 1) * P].rearrange(
                                "(kt p) m -> p kt m", p=P))
                        ps = wo_ps.tile([P, 512], f32, tag="wops")
                        for k in range(NH2):
                            nc.tensor.matmul(ps, slab[:, k, :], rhs[:, k, :],
                                             start=(k == 0),
                                             stop=(k == NH2 - 1))
                        if half == 0:
                            nc.vector.tensor_copy(acc[:, m, :], ps)
                        else:
                            ev = wo_ev.tile([P, 512], f32, tag="woev")
                            nc.vector.tensor_add(ev, ps, acc[:, m, :])
                            nc.sync.dma_start(out=outT[m * P:(m + 1) * P, :],
                                              in_=ev)

    nc.compile()
    return nc


def _prep_inputs(x, freqs_cos, freqs_sin, wq_a, q_norm_w, wq_b, wkv_a,
                 kv_norm_w, wkv_b, wo):
    x = np.asarray(x, np.float32)
    freqs_cos = np.asarray(freqs_cos, np.float32)
    freqs_sin = np.asarray(freqs_sin, np.float32)
    wq_a = np.asarray(wq_a, np.float32)
    q_norm_w = np.asarray(q_norm_w, np.float32)
    wq_b = np.asarray(wq_b, np.float32)
    wkv_a = np.asarray(wkv_a, np.float32)
    kv_norm_w = np.asarray(kv_norm_w, np.float32)
    wkv_b = np.asarray(wkv_b, np.float32)
    wo = np.asarray(wo, np.float32)

    wqaT = np.ascontiguousarray(wq_a.T).astype(BF16)
    wkvaT = np.ascontiguousarray(wkv_a.T).astype(BF16)

    wqb_eff = (wq_b * q_norm_w[None, :]) * SCALE
    wqb_eff = wqb_eff.reshape(N_HEADS, QK_HD, Q_LORA)
    wkvb_eff = wkv_b * kv_norm_w[None, :]
    wkvb_eff = wkvb_eff.reshape(N_HEADS, NOPE + V_DIM, KV_LORA)

    cosT = np.repeat(freqs_cos.T, 2, axis=0)  # [64, S]
    sinT = np.repeat(freqs_sin.T, 2, axis=0)

    perm = np.zeros((ROPE, ROPE), np.float32)
    for i in range(ROPE // 2):
        perm[2 * i + 1, 2 * i] = -1.0  # out[2i]   = -x[2i+1]
        perm[2 * i, 2 * i + 1] = 1.0   # out[2i+1] =  x[2i]
    r = np.arange(P)
    trimask = np.where(r[:, None] <= r[None, :], 0.0,
                       -1e30).astype(np.float32)

    # wo rows reordered to match the head-pair gathers:
    # o_g[0] rows (per sender g') = heads {4g', 4g'+1}; o_g[1] = {4g'+2,+3}
    woT = wo.T.reshape(N_HEADS // 4, 4, V_DIM, DIM)  # [g', local h, 128, D]
    woTa = woT[:, :2].reshape(-1, DIM)
    woTb = woT[:, 2:].reshape(-1, DIM)

    in_maps = []
    for c in range(NCORES):
        b, g = c // GROUP, c % GROUP
        heads = slice(g * HPC, (g + 1) * HPC)
        xTc = np.ascontiguousarray(
            x[b].T[:, g * SSH:(g + 1) * SSH]).astype(BF16)
        wqbT = np.concatenate(
            [wqb_eff[heads, :NOPE].reshape(HPC * NOPE, Q_LORA),
             wqb_eff[heads, NOPE:].reshape(HPC * ROPE, Q_LORA)],
            axis=0).T
        wkvbT = np.concatenate(
            [wkvb_eff[heads, :NOPE].reshape(HPC * NOPE, KV_LORA),
             wkvb_eff[heads, NOPE:].reshape(HPC * V_DIM, KV_LORA)],
            axis=0).T
        in_maps.append({
            "xT": xTc,
            "wqaT": wqaT,
            "wkvaT": wkvaT,
            "wqbT": np.ascontiguousarray(wqbT).astype(BF16),
            "wkvbT": np.ascontiguousarray(wkvbT).astype(BF16),
            "woTa": np.ascontiguousarray(woTa).astype(BF16),
            "woTb": np.ascontiguousarray(woTb).astype(BF16),
            "cos_sh": np.ascontiguousarray(
                cosT[:, g * SSH:(g + 1) * SSH]).astype(BF16),
            "sin_sh": np.ascontiguousarray(
                sinT[:, g * SSH:(g + 1) * SSH]).astype(BF16),
            "cos_full": np.ascontiguousarray(cosT).astype(BF16),
            "sin_full": np.ascontiguousarray(sinT).astype(BF16),
            "perm64": perm.astype(BF16),
            "trimask": trimask,
            "cfg": np.array([[b * GROUP * KVROWS, b * GROUP * QROWS,
                              b * (S // 2), g * 512]], np.int32),
        })
    return in_maps


def _run(inputs, trace=False, **kw):
    if "nc" not in _cache:
        _cache["nc"] = _build()
    nc = _cache["nc"]
    in_maps = _prep_inputs(**inputs)
    res = run_bass_kernel_spmd(nc, in_maps, core_ids=list(range(NCORES)),
                               trace=trace, **kw)
    out = np.empty((B, S, DIM), np.float32)
    for c in range(NCORES):
        b, g = c // GROUP, c % GROUP
        out[b, g * SSH:(g + 1) * SSH, :] = res.results[c]["out"].T
    return out, res


def kernel(**inputs):
    out, _ = _run(inputs)
    return out
